# revision 29
# baseline (speedup 1.0000x reference)
"""Trainium2 Bass kernel for the SEAN/SPADE-style normalization module.

Strategy (8 NeuronCores = batch(2) x output-channel-slice(4 x 128)):
  * The scatter `middle_avg[b,:,h,w] = mu_codes[b, label(h,w), :]` makes
    middle_avg rank-19, so conv3x3(middle_avg, W) == conv3x3(segmap, W_eff)
    with W_eff[c,j,dd] = sum_d W[c,d,dd] * mu_codes[b,j,d].  The two big
    512->512 convs collapse into 19->512 convs with tiny on-device
    effective-weight matmuls.
  * The one-hot segmap is shipped as THREE column-shifted copies packed on
    the partition axis (kx baked into the copy index), so every seg-input
    conv needs only 3 matmuls per pixel tile (one per ky) instead of 9.
  * Per-class FC (style_codes @ W_mu) runs fully per-core (W_mu streamed as
    bf16, one class per mix-loop iteration) - no collectives, no cross-core
    coupling.
  * Instance-norm is fully local per core (full spatial extent present).
    Noise is added on device via a k=1 PE outer product (noise_var x noise_row).
  * Blend factors sigmoid(blending_*) are folded into the conv weights on the
    host, so gamma_avg/gamma_spade accumulate into the same PSUM bank and the
    epilogue is normalized*(1+gamma) + beta with per-partition affine ops.

Convs are computed in bf16 (fp32 PSUM accumulation); norm + epilogue in fp32.
"""

import sys

for _p in ("/opt/trn_rl_repo",):
    if _p not in sys.path:
        sys.path.insert(0, _p)

import numpy as np
import ml_dtypes

import concourse.bass as bass
import concourse.mybir as mybir
from concourse import bacc
from concourse.bass_utils import run_bass_kernel_spmd
import concourse.tile as tile

F32 = mybir.dt.float32
BF16 = mybir.dt.bfloat16

B, C, H, W = 2, 512, 128, 128
NCLS, SL, NHID = 19, 512, 128
P = 128
NCORES = 8
HP = WP = H + 2
NPIX = H * W
NT = NPIX // 512   # 32 pixel tiles of 4 rows x 128 cols
EPS = 1e-5
KREP = 96          # 3 shifted seg copies x 32 rows

_cache = {}


def _build():
    nc = bacc.Bacc("TRN2", target_bir_lowering=False, debug=False,
                   num_devices=NCORES)

    def din(name, shape, dt=F32):
        return nc.dram_tensor(name, list(shape), dt, kind="ExternalInput")

    x = din("x_s", [P, NPIX])
    nt = din("nt", [1, NPIX], BF16)      # noise row-map, stays in DRAM
    nv = din("nv", [1, P], BF16)
    seg = din("seg", [P, HP * WP], BF16)
    wmu = din("wmu", [NCLS, P, 4, SL], BF16)     # [j][c%128, cq, d]
    style = din("style", [P, NCLS, 4, 1], BF16)  # [c%128, j, cq, 1]
    bmu = din("bmu", [32, SL])                   # [j, d] rows 19+ zero
    idn = din("idn", [32, 32], BF16)             # identity for PE transpose
    wg = din("wg", [P, 9, 4, P], BF16)
    wb = din("wb", [P, 9, 4, P], BF16)
    wsh = din("wsh", [P, 3, NHID], BF16)
    bsh = din("bsh", [NHID, 1])
    wsg = din("wsg", [NHID, 9, P], BF16)
    wsb = din("wsb", [NHID, 9, P], BF16)
    gbias = din("gbias", [P, 1])
    bbias = din("bbias", [P, 1])
    out = nc.dram_tensor("out", [P, NPIX], F32, kind="ExternalOutput")
    mu_dram = nc.dram_tensor("mu_dram", [NCLS, SL], F32)

    def rep_ap(t, ky, buf):
        """rhs AP into the 3x-shifted replicated segmap (kx baked per copy),
        pixel tile t (rows 4t..4t+3)."""
        src = buf[:, (4 * t + ky) * WP:]
        return bass.AP(tensor=src.tensor, offset=src.offset,
                       ap=[src.ap[0], [WP, 4], [1, W]])

    def pix_ap(t, ky, kx, buf):
        src = buf[:, (4 * t + ky) * WP + kx:]
        return bass.AP(tensor=src.tensor, offset=src.offset,
                       ap=[src.ap[0], [WP, 4], [1, W]])

    offs = [(ky, kx) for ky in range(3) for kx in range(3)]

    with tile.TileContext(nc) as tc:
        with (
            tc.tile_pool(name="big", bufs=1) as big,
            tc.tile_pool(name="cst", bufs=1) as cst,
            tc.tile_pool(name="ntp", bufs=8) as ntp,
            tc.tile_pool(name="stp", bufs=1) as stp,
        ):
            # ---- input DMAs: seg/wsh first (they gate the first matmuls)
            segb = big.tile([P, HP * WP], BF16)
            for qq in range(4):
                sl0 = qq * (HP * WP // 4)
                sl1 = (HP * WP) if qq == 3 else (qq + 1) * (HP * WP // 4)
                nc.sync.dma_start(out=segb[:, sl0:sl1], in_=seg.ap()[:, sl0:sl1])
            wshb = cst.tile([P, 3, NHID], BF16)
            nc.sync.dma_start(out=wshb[:], in_=wsh.ap())
            bshb = cst.tile([NHID, 1], F32)
            nc.sync.dma_start(out=bshb[:], in_=bsh.ap())
            nvb = cst.tile([1, P], BF16)
            nc.sync.dma_start(out=nvb[:], in_=nv.ap())

            xq = []
            for qq in range(4):
                xt = big.tile([P, NPIX // 4], F32, tag=f"xq{qq}")
                if qq == 0:
                    nc.sync.dma_start(
                        out=xt[:],
                        in_=x.ap()[:, qq * (NPIX // 4):(qq + 1) * (NPIX // 4)])
                xq.append(xt)
            _x_sched = {2: 1, 9: 2, 17: 3}  # emit quarter q at these tiles

            def xsl(t):
                return xq[t // 8][:, (t % 8) * 512:(t % 8) * 512 + 512]

            actv = big.tile([P, HP * WP], BF16)
            wsgb = cst.tile([NHID, 9, P], BF16)
            nc.sync.dma_start(out=wsgb[:], in_=wsg.ap())
            wsbb = cst.tile([NHID, 9, P], BF16)
            nc.sync.dma_start(out=wsbb[:], in_=wsb.ap())
            gbb = cst.tile([P, 1], F32)
            nc.sync.dma_start(out=gbb[:], in_=gbias.ap())
            bbb = cst.tile([P, 1], F32)
            nc.sync.dma_start(out=bbb[:], in_=bbias.ap())
            idnb = cst.tile([32, 32], BF16)
            nc.sync.dma_start(out=idnb[:], in_=idn.ap())
            weff_g = cst.tile([P, 3, P], BF16)
            nc.vector.memset(weff_g[:], 0.0)
            weff_b = cst.tile([P, 3, P], BF16)
            nc.vector.memset(weff_b[:], 0.0)

            # actv border zeros (interior is fully written by the relu copies)
            nc.vector.memset(actv[:, 0:WP], 0.0)
            nc.vector.memset(actv[:, (HP - 1) * WP:HP * WP], 0.0)
            a0 = actv[:, 0:]
            nc.vector.memset(bass.AP(tensor=a0.tensor, offset=a0.offset + WP,
                                     ap=[a0.ap[0], [WP, H], [1, 1]]), 0.0)
            nc.vector.memset(bass.AP(tensor=a0.tensor,
                                     offset=a0.offset + WP + (WP - 1),
                                     ap=[a0.ap[0], [WP, H], [1, 1]]), 0.0)

            fcw_cm = tc.tile_pool(name="fcw", bufs=2)
            fcw = fcw_cm.__enter__()

            # static FC inputs
            fcc_cm = tc.tile_pool(name="fcc", bufs=1)
            fcc = fcc_cm.__enter__()
            styb = fcc.tile([P, NCLS, 4, 1], BF16)
            nc.sync.dma_start(out=styb[:], in_=style.ap())
            bmub = fcc.tile([32, SL], F32)
            nc.sync.dma_start(out=bmub[:], in_=bmu.ap())
            mu_flat = fcc.tile([32, SL], F32)

            # ---------------- Phase 1: mix loop - SPADE hidden conv + noise
            # outer-products + bn_stats, with one FC class interleaved per
            # iteration (streams W_mu while the PE stays dense)
            stats = stp.tile([P, NT, 6], F32)
            _fc_sched = {round(j * 26 / (NCLS - 1)): j for j in range(NCLS)}
            assert len(_fc_sched) == NCLS
            psfc_cm = tc.tile_pool(name="psfc", bufs=2, space="PSUM")
            psfc = psfc_cm.__enter__()
            with (
                tc.tile_pool(name="psa", bufs=3, space="PSUM") as psa,
                tc.tile_pool(name="psn", bufs=3, space="PSUM") as psn,
            ):
                for t in range(NT):
                    pa = psa.tile([NHID, 512], F32, tag="pa")
                    for ky in range(3):
                        nc.tensor.matmul(pa[:], lhsT=wshb[:, ky, :],
                                         rhs=rep_ap(t, ky, segb),
                                         start=(ky == 0), stop=(ky == 2))
                    dst = actv[:, (4 * t + 1) * WP + 1:]
                    dst = bass.AP(tensor=dst.tensor, offset=dst.offset,
                                  ap=[dst.ap[0], [WP, 4], [1, W]])
                    nc.scalar.activation(out=dst, in_=pa[:],
                                         func=mybir.ActivationFunctionType.Relu,
                                         bias=bshb[:], scale=1.0)
                    # noise outer product for this tile
                    ntt = ntp.tile([1, 512], BF16, tag="ntt")
                    nc.sync.dma_start(out=ntt[:],
                                      in_=nt.ap()[:, 512 * t:512 * (t + 1)])
                    nps = psn.tile([P, 512], F32, tag="nps")
                    nc.tensor.matmul(nps[:], lhsT=nvb[:], rhs=ntt[:],
                                     start=True, stop=True)
                    nc.vector.tensor_add(xsl(t), xsl(t), nps[:])
                    nc.vector.bn_stats(stats[:, t, :], xsl(t))
                    if t in _x_sched:
                        qq = _x_sched[t]
                        nc.sync.dma_start(
                            out=xq[qq][:],
                            in_=x.ap()[:, qq * (NPIX // 4):(qq + 1) * (NPIX // 4)])
                    # FC (streams its W_mu block), classes spread over all
                    # 32 tiles to smooth HBM demand; style is the (1-column)
                    # stationary operand so there is no per-class LDW cost
                    if t in _fc_sched:
                        j = _fc_sched[t]
                        wmt = fcw.tile([P, 4, SL], BF16, tag="wmt")
                        nc.sync.dma_start(out=wmt[:], in_=wmu.ap()[j])
                        mfc = psfc.tile([1, SL], F32, tag="mfc")
                        for kq in range(4):
                            nc.tensor.matmul(
                                mfc[:], lhsT=styb[:, j, kq, :],
                                rhs=wmt[:, kq, :],
                                start=(kq == 0), stop=(kq == 3))
                        mstg = fcw.tile([1, SL], F32, tag="mstg")
                        nc.scalar.activation(
                            out=mstg[:], in_=mfc[:],
                            func=mybir.ActivationFunctionType.Identity,
                            bias=0.0, scale=1.0)
                        nc.gpsimd.dma_start(out=mu_dram.ap()[j:j + 1, :],
                                            in_=mstg[:])
                    if t == 20:
                        wgb = fcc.tile([P, 9, 4, P], BF16)
                        nc.sync.dma_start(out=wgb[:], in_=wg.ap())
                    if t == 24:
                        wbb_ = fcc.tile([P, 9, 4, P], BF16)
                        nc.sync.dma_start(out=wbb_[:], in_=wb.ap())

            # ---------------- Phase 2: finalize stats
            mv = stp.tile([P, 2], F32)
            nc.vector.bn_aggr(mv[:], stats[:])
            epsb = stp.tile([P, 1], F32)
            nc.vector.memset(epsb[:], EPS)
            rs = stp.tile([P, 1], F32)
            nc.scalar.activation(out=rs[:], in_=mv[:, 1:2],
                                 func=mybir.ActivationFunctionType.Sqrt,
                                 bias=epsb[:], scale=1.0)
            nc.vector.reciprocal(rs[:], rs[:])
            nbias = stp.tile([P, 1], F32)
            nc.vector.tensor_mul(nbias[:], mv[:, 0:1], rs[:])
            nc.vector.tensor_scalar_mul(nbias[:], nbias[:], -1.0)

            # ---------------- Phase 3: mu = relu(fc + bias), transpose to
            # the d-partitioned layout via 4 PE transposes
            nc.gpsimd.dma_start(out=mu_flat[:NCLS, :], in_=mu_dram.ap())
            nc.gpsimd.tensor_add(mu_flat[:NCLS, :], mu_flat[:NCLS, :],
                                 bmub[:NCLS, :])
            nc.gpsimd.tensor_scalar_max(mu_flat[:NCLS, :],
                                        mu_flat[:NCLS, :], 0.0)
            mu_fbf = fcc.tile([32, SL], BF16)
            nc.gpsimd.memset(mu_fbf[:], 0.0)
            nc.gpsimd.tensor_copy(mu_fbf[:NCLS, :], mu_flat[:NCLS, :])
            mu_bf = fcc.tile([P, 4, NCLS], BF16)
            psfc_cm.__exit__(None, None, None)
            with tc.tile_pool(name="pst", bufs=2, space="PSUM") as pst:
                for q in range(4):
                    psT = pst.tile([P, 32], BF16, tag="psT")
                    nc.tensor.transpose(psT[:], mu_fbf[:, q * P:(q + 1) * P],
                                        idnb[:])
                    nc.vector.tensor_copy(mu_bf[:, q, :], psT[:, :NCLS])

            # ---------------- Phase 4: effective weights
            with tc.tile_pool(name="pse", bufs=2, space="PSUM") as pse:
                for wsrcb, wdst in ((wgb, weff_g), (wbb_, weff_b)):
                    pe = pse.tile([NCLS, 9, P], F32, tag="pse")
                    for dd in range(9):
                        for q in range(4):
                            nc.tensor.matmul(pe[:, dd, :], lhsT=mu_bf[:, q, :],
                                             rhs=wsrcb[:, dd, q, :],
                                             start=(q == 0), stop=(q == 3))
                    for c in range(3):
                        src = pe[:, c:, :]
                        src = bass.AP(tensor=src.tensor, offset=src.offset,
                                      ap=[src.ap[0], [3 * P, 3], [1, P]])
                        nc.vector.tensor_copy(
                            wdst[32 * c:32 * c + NCLS, :, :], src)
            fcc_cm.__exit__(None, None, None)
            fcw_cm.__exit__(None, None, None)

            # ---------------- Phase 5: main conv + epilogue
            with (
                tc.tile_pool(name="psm", bufs=4, space="PSUM") as psm,
                tc.tile_pool(name="wk", bufs=3) as wk,
            ):
                for t in range(NT):
                    gps = psm.tile([P, 512], F32, tag="gps")
                    bps = psm.tile([P, 512], F32, tag="bps")
                    for ky in range(3):
                        nc.tensor.matmul(gps[:], lhsT=weff_g[:KREP, ky, :],
                                         rhs=rep_ap(t, ky, segb[:KREP]),
                                         start=(ky == 0), stop=False)
                    for i, (ky, kx) in enumerate(offs):
                        nc.tensor.matmul(gps[:], lhsT=wsgb[:, i, :],
                                         rhs=pix_ap(t, ky, kx, actv),
                                         start=False, stop=(i == 8))
                    for ky in range(3):
                        nc.tensor.matmul(bps[:], lhsT=weff_b[:KREP, ky, :],
                                         rhs=rep_ap(t, ky, segb[:KREP]),
                                         start=(ky == 0), stop=False)
                    for i, (ky, kx) in enumerate(offs):
                        nc.tensor.matmul(bps[:], lhsT=wsbb[:, i, :],
                                         rhs=pix_ap(t, ky, kx, actv),
                                         start=False, stop=(i == 8))
                    sl = slice(512 * t, 512 * (t + 1))
                    nrm = wk.tile([P, 512], F32, tag="nrm")
                    nc.scalar.activation(out=nrm[:], in_=xsl(t),
                                         func=mybir.ActivationFunctionType.Identity,
                                         bias=nbias[:], scale=rs[:])
                    g = wk.tile([P, 512], F32, tag="g")
                    nc.scalar.activation(out=g[:], in_=gps[:],
                                         func=mybir.ActivationFunctionType.Identity,
                                         bias=gbb[:], scale=1.0)
                    bb2 = wk.tile([P, 512], F32, tag="bb2")
                    nc.scalar.activation(out=bb2[:], in_=bps[:],
                                         func=mybir.ActivationFunctionType.Identity,
                                         bias=bbb[:], scale=1.0)
                    o = wk.tile([P, 512], F32, tag="o")
                    nc.vector.tensor_mul(o[:], nrm[:], g[:])
                    nc.vector.tensor_add(o[:], o[:], bb2[:])
                    nc.sync.dma_start(out=out.ap()[:, sl], in_=o[:])

    nc.compile()
    return nc


def _prep_inputs(x, segmap, style_codes, noise, noise_var, blending_gamma,
                 blending_beta, W_mu, b_mu, conv_gamma_w, conv_gamma_b,
                 conv_beta_w, conv_beta_b, sp_shared_w, sp_shared_b,
                 sp_gamma_w, sp_gamma_b, sp_beta_w, sp_beta_b):
    f32 = np.float32
    bf = ml_dtypes.bfloat16
    x = np.asarray(x, f32)
    segmap = np.asarray(segmap, f32)
    style_codes = np.asarray(style_codes, f32)
    noise = np.asarray(noise, f32)
    noise_var = np.asarray(noise_var, f32)
    W_mu = np.asarray(W_mu, f32)
    b_mu = np.asarray(b_mu, f32)

    ga = float(1.0 / (1.0 + np.exp(-f32(np.asarray(blending_gamma).reshape(-1)[0]))))
    ba = float(1.0 / (1.0 + np.exp(-f32(np.asarray(blending_beta).reshape(-1)[0]))))

    # padded one-hot segmap, replicated into 3 column-shifted copies on the
    # partition axis (kx baked into copy index), bf16 (exact: values 0/1)
    tmp = np.zeros((B, NCLS, HP, WP), bf)
    tmp[:, :, 1:H + 1, 1:W + 1] = segmap.astype(bf)
    tmp = tmp.reshape(B, NCLS, HP * WP)
    segp = np.zeros((B, P, HP * WP), bf)
    for c in range(3):
        segp[:, 32 * c:32 * c + NCLS, :HP * WP - c] = tmp[:, :, c:]

    # noise row-map per batch: nt[h, w] = noise[b, w, h, 0]
    ntm = noise[:, :, :, 0].transpose(0, 2, 1).reshape(B, 1, NPIX).astype(bf)

    # FC weights: wmu_t[j, p, kq, d] = W_mu[j, kq*128+p, d]
    wmu_t = np.ascontiguousarray(
        W_mu.reshape(NCLS, 4, P, SL).transpose(0, 2, 1, 3).astype(bf))
    # style_t[b][p, j, kq, 1] = style_codes[b, j, kq*128+p]
    style_t = style_codes.reshape(B, NCLS, 4, P).transpose(0, 3, 1, 2)[..., None].astype(bf)
    style_t = [np.ascontiguousarray(style_t[b]) for b in range(B)]
    # bmu_t[j, d] = b_mu[j, d], rows 19+ zero
    bmu_t = np.zeros((32, SL), f32)
    bmu_t[:NCLS] = b_mu
    idn_t = np.eye(32, dtype=bf)

    # big conv weights -> [dd, d, c] (blend factors folded in)
    wgt = (ga * conv_gamma_w.astype(f32)).transpose(2, 3, 1, 0).reshape(9, SL, C)
    wbt = (ba * conv_beta_w.astype(f32)).transpose(2, 3, 1, 0).reshape(9, SL, C)

    # SPADE weights; wsh replicated: row 32c+j, col ky = w[n, j, ky, c]
    wsh_t = np.zeros((P, 3, NHID), bf)
    _w = np.asarray(sp_shared_w, f32).transpose(1, 2, 3, 0)  # [j, ky, kx, n]
    for c in range(3):
        wsh_t[32 * c:32 * c + NCLS] = _w[:, :, c, :].astype(bf)
    wsg_t = ((1.0 - ga) * np.asarray(sp_gamma_w, f32)).transpose(1, 2, 3, 0).reshape(NHID, 9, C)
    wsb_t = ((1.0 - ba) * np.asarray(sp_beta_w, f32)).transpose(1, 2, 3, 0).reshape(NHID, 9, C)

    gbias_full = (1.0 + ga * np.asarray(conv_gamma_b, f32)
                  + (1.0 - ga) * np.asarray(sp_gamma_b, f32))
    bbias_full = (ba * np.asarray(conv_beta_b, f32)
                  + (1.0 - ba) * np.asarray(sp_beta_b, f32))
    bsh_t = np.asarray(sp_shared_b, f32).reshape(NHID, 1)

    in_maps = []
    for core in range(NCORES):
        b = core // 4
        cs = core % 4
        csl = slice(cs * P, (cs + 1) * P)
        m = {
            "x_s": np.ascontiguousarray(x[b, csl].reshape(P, NPIX)),
            "nt": ntm[b],
            "nv": np.ascontiguousarray(noise_var[csl].reshape(1, P).astype(bf)),
            "seg": np.ascontiguousarray(segp[b]),
            "wmu": wmu_t,
            "style": style_t[b],
            "bmu": bmu_t,
            "idn": idn_t,
            "wg": np.ascontiguousarray(
                wgt[:, :, csl].reshape(9, 4, P, P).transpose(2, 0, 1, 3).astype(bf)),
            "wb": np.ascontiguousarray(
                wbt[:, :, csl].reshape(9, 4, P, P).transpose(2, 0, 1, 3).astype(bf)),
            "wsh": wsh_t,
            "bsh": bsh_t,
            "wsg": np.ascontiguousarray(wsg_t[:, :, csl].astype(bf)),
            "wsb": np.ascontiguousarray(wsb_t[:, :, csl].astype(bf)),
            "gbias": np.ascontiguousarray(gbias_full[csl].reshape(P, 1)),
            "bbias": np.ascontiguousarray(bbias_full[csl].reshape(P, 1)),
        }
        in_maps.append(m)
    return in_maps


def kernel(**inputs):
    if "nc" not in _cache:
        _cache["nc"] = _build()
    nc = _cache["nc"]
    in_maps = _prep_inputs(**inputs)
    res = run_bass_kernel_spmd(nc, in_maps, core_ids=list(range(NCORES)))
    out = np.empty((B, C, H, W), np.float32)
    for core in range(NCORES):
        b = core // 4
        cs = core % 4
        out[b, cs * P:(cs + 1) * P] = np.asarray(
            res.results[core]["out"]).reshape(P, H, W)
    return out


# revision 30
# speedup vs baseline: 1.2273x; 1.2273x over previous
"""Trainium2 Bass kernel for the SEAN/SPADE-style normalization module.

Strategy (8 NeuronCores = batch(2) x output-channel-slice(4 x 128)):
  * The scatter `middle_avg[b,:,h,w] = mu_codes[b, label(h,w), :]` makes
    middle_avg rank-19, so conv3x3(middle_avg, W) == conv3x3(segmap, W_eff)
    with W_eff[c,j,dd] = sum_d W[c,d,dd] * mu_codes[b,j,d].  The two big
    512->512 convs collapse into 19->512 convs with tiny on-device
    effective-weight matmuls.
  * The one-hot segmap is shipped as THREE column-shifted copies packed on
    the partition axis (kx baked into the copy index), so every seg-input
    conv needs only 3 matmuls per pixel tile (one per ky) instead of 9.
  * Per-class FC (style_codes @ W_mu) runs fully per-core (W_mu streamed as
    bf16, one class per mix-loop iteration) - no collectives, no cross-core
    coupling.
  * Instance-norm is fully local per core (full spatial extent present).
    Noise is added on device via a k=1 PE outer product (noise_var x noise_row).
  * Blend factors sigmoid(blending_*) are folded into the conv weights on the
    host, so gamma_avg/gamma_spade accumulate into the same PSUM bank and the
    epilogue is normalized*(1+gamma) + beta with per-partition affine ops.

Convs are computed in bf16 (fp32 PSUM accumulation); norm + epilogue in fp32.
"""

import sys

for _p in ("/opt/trn_rl_repo",):
    if _p not in sys.path:
        sys.path.insert(0, _p)

import numpy as np
import ml_dtypes

import concourse.bass as bass
import concourse.mybir as mybir
from concourse import bacc
from concourse.bass_utils import run_bass_kernel_spmd
import concourse.tile as tile

F32 = mybir.dt.float32
BF16 = mybir.dt.bfloat16

B, C, H, W = 2, 512, 128, 128
NCLS, SL, NHID = 19, 512, 128
P = 128
NCORES = 8
HP = WP = H + 2
NPIX = H * W
NT = NPIX // 512   # 32 pixel tiles of 4 rows x 128 cols
EPS = 1e-5
KREP = 96          # 3 shifted seg copies x 32 rows

_cache = {}


def _build():
    nc = bacc.Bacc("TRN2", target_bir_lowering=False, debug=False,
                   num_devices=NCORES)

    def din(name, shape, dt=F32):
        return nc.dram_tensor(name, list(shape), dt, kind="ExternalInput")

    x = din("x_s", [P, NPIX])
    nt = din("nt", [1, NPIX], BF16)      # noise row-map, stays in DRAM
    nv = din("nv", [1, P], BF16)
    seg = din("seg", [P, HP * WP], BF16)
    wmu = din("wmu", [NCLS, P, 4, SL], BF16)     # [j][c%128, cq, d]
    style = din("style", [P, NCLS, 4, 1], BF16)  # [c%128, j, cq, 1]
    bmu = din("bmu", [32, SL])                   # [j, d] rows 19+ zero
    idn = din("idn", [32, 32], BF16)             # identity for PE transpose
    wg = din("wg", [P, 9, 4, P], BF16)
    wb = din("wb", [P, 9, 4, P], BF16)
    wsh = din("wsh", [P, 3, NHID], BF16)
    bsh = din("bsh", [NHID, 1])
    wsg = din("wsg", [NHID, 9, P], BF16)
    wsb = din("wsb", [NHID, 9, P], BF16)
    gbias = din("gbias", [P, 1])
    bbias = din("bbias", [P, 1])
    out = nc.dram_tensor("out", [P, NPIX], F32, kind="ExternalOutput")
    mu_dram = nc.dram_tensor("mu_dram", [NCLS, SL], F32)

    def rep_ap(t, ky, buf):
        """rhs AP into the 3x-shifted replicated segmap (kx baked per copy),
        pixel tile t (rows 4t..4t+3)."""
        src = buf[:, (4 * t + ky) * WP:]
        return bass.AP(tensor=src.tensor, offset=src.offset,
                       ap=[src.ap[0], [WP, 4], [1, W]])

    def pix_ap(t, ky, kx, buf):
        src = buf[:, (4 * t + ky) * WP + kx:]
        return bass.AP(tensor=src.tensor, offset=src.offset,
                       ap=[src.ap[0], [WP, 4], [1, W]])

    offs = [(ky, kx) for ky in range(3) for kx in range(3)]

    with tile.TileContext(nc) as tc:
        with (
            tc.tile_pool(name="big", bufs=1) as big,
            tc.tile_pool(name="cst", bufs=1) as cst,
            tc.tile_pool(name="ntp", bufs=8) as ntp,
            tc.tile_pool(name="stp", bufs=1) as stp,
        ):
            # ---- input DMAs: seg/wsh first (they gate the first matmuls)
            segb = big.tile([P, HP * WP], BF16)
            for qq in range(4):
                sl0 = qq * (HP * WP // 4)
                sl1 = (HP * WP) if qq == 3 else (qq + 1) * (HP * WP // 4)
                nc.sync.dma_start(out=segb[:, sl0:sl1], in_=seg.ap()[:, sl0:sl1])
            wshb = cst.tile([P, 3, NHID], BF16)
            nc.sync.dma_start(out=wshb[:], in_=wsh.ap())
            bshb = cst.tile([NHID, 1], F32)
            nc.sync.dma_start(out=bshb[:], in_=bsh.ap())
            nvb = cst.tile([1, P], BF16)
            nc.sync.dma_start(out=nvb[:], in_=nv.ap())

            xq = []
            for qq in range(4):
                xt = big.tile([P, NPIX // 4], F32, tag=f"xq{qq}")
                if qq == 0:
                    nc.sync.dma_start(
                        out=xt[:],
                        in_=x.ap()[:, qq * (NPIX // 4):(qq + 1) * (NPIX // 4)])
                xq.append(xt)
            _x_sched = {2: 1, 9: 2, 17: 3}  # emit quarter q at these tiles

            def xsl(t):
                return xq[t // 8][:, (t % 8) * 512:(t % 8) * 512 + 512]

            actv = big.tile([P, HP * WP], BF16)
            wsgb = cst.tile([NHID, 9, P], BF16)
            nc.sync.dma_start(out=wsgb[:], in_=wsg.ap())
            wsbb = cst.tile([NHID, 9, P], BF16)
            nc.sync.dma_start(out=wsbb[:], in_=wsb.ap())
            gbb = cst.tile([P, 1], F32)
            nc.sync.dma_start(out=gbb[:], in_=gbias.ap())
            bbb = cst.tile([P, 1], F32)
            nc.sync.dma_start(out=bbb[:], in_=bbias.ap())
            idnb = cst.tile([32, 32], BF16)
            nc.sync.dma_start(out=idnb[:], in_=idn.ap())
            weff_g = cst.tile([P, 3, P], BF16)
            nc.vector.memset(weff_g[:], 0.0)
            weff_b = cst.tile([P, 3, P], BF16)
            nc.vector.memset(weff_b[:], 0.0)

            # actv border zeros (interior is fully written by the relu copies)
            nc.vector.memset(actv[:, 0:WP], 0.0)
            nc.vector.memset(actv[:, (HP - 1) * WP:HP * WP], 0.0)
            a0 = actv[:, 0:]
            nc.vector.memset(bass.AP(tensor=a0.tensor, offset=a0.offset + WP,
                                     ap=[a0.ap[0], [WP, H], [1, 1]]), 0.0)
            nc.vector.memset(bass.AP(tensor=a0.tensor,
                                     offset=a0.offset + WP + (WP - 1),
                                     ap=[a0.ap[0], [WP, H], [1, 1]]), 0.0)

            fcw_cm = tc.tile_pool(name="fcw", bufs=2)
            fcw = fcw_cm.__enter__()

            # static FC inputs
            fcc_cm = tc.tile_pool(name="fcc", bufs=1)
            fcc = fcc_cm.__enter__()
            styb = fcc.tile([P, NCLS, 4, 1], BF16)
            nc.sync.dma_start(out=styb[:], in_=style.ap())
            bmub = fcc.tile([32, SL], F32)
            nc.sync.dma_start(out=bmub[:], in_=bmu.ap())
            mu_flat = fcc.tile([32, SL], F32)

            # ---------------- Phase 1: mix loop - SPADE hidden conv + noise
            # outer-products + bn_stats, with one FC class interleaved per
            # iteration (streams W_mu while the PE stays dense)
            stats = stp.tile([P, NT, 6], F32)
            _fc_sched = {round(j * 26 / (NCLS - 1)): j for j in range(NCLS)}
            assert len(_fc_sched) == NCLS
            psfc_cm = tc.tile_pool(name="psfc", bufs=2, space="PSUM")
            psfc = psfc_cm.__enter__()
            with (
                tc.tile_pool(name="psa", bufs=3, space="PSUM") as psa,
                tc.tile_pool(name="psn", bufs=3, space="PSUM") as psn,
            ):
                for t in range(NT):
                    pa = psa.tile([NHID, 512], F32, tag="pa")
                    for ky in range(3):
                        nc.tensor.matmul(pa[:], lhsT=wshb[:, ky, :],
                                         rhs=rep_ap(t, ky, segb),
                                         start=(ky == 0), stop=(ky == 2))
                    dst = actv[:, (4 * t + 1) * WP + 1:]
                    dst = bass.AP(tensor=dst.tensor, offset=dst.offset,
                                  ap=[dst.ap[0], [WP, 4], [1, W]])
                    nc.scalar.activation(out=dst, in_=pa[:],
                                         func=mybir.ActivationFunctionType.Relu,
                                         bias=bshb[:], scale=1.0)
                    # noise outer product for this tile
                    ntt = ntp.tile([1, 512], BF16, tag="ntt")
                    nc.sync.dma_start(out=ntt[:],
                                      in_=nt.ap()[:, 512 * t:512 * (t + 1)])
                    nps = psn.tile([P, 512], F32, tag="nps")
                    nc.tensor.matmul(nps[:], lhsT=nvb[:], rhs=ntt[:],
                                     start=True, stop=True)
                    nc.vector.tensor_add(xsl(t), xsl(t), nps[:])
                    nc.vector.bn_stats(stats[:, t, :], xsl(t))
                    if t in _x_sched:
                        qq = _x_sched[t]
                        nc.sync.dma_start(
                            out=xq[qq][:],
                            in_=x.ap()[:, qq * (NPIX // 4):(qq + 1) * (NPIX // 4)])
                    # FC (streams its W_mu block), classes spread over all
                    # 32 tiles to smooth HBM demand; style is the (1-column)
                    # stationary operand so there is no per-class LDW cost
                    if t in _fc_sched:
                        j = _fc_sched[t]
                        wmt = fcw.tile([P, 4, SL], BF16, tag="wmt")
                        nc.sync.dma_start(out=wmt[:], in_=wmu.ap()[j])
                        mfc = psfc.tile([1, SL], F32, tag="mfc")
                        for kq in range(4):
                            nc.tensor.matmul(
                                mfc[:], lhsT=styb[:, j, kq, :],
                                rhs=wmt[:, kq, :],
                                start=(kq == 0), stop=(kq == 3))
                        mstg = fcw.tile([1, SL], F32, tag="mstg")
                        nc.scalar.activation(
                            out=mstg[:], in_=mfc[:],
                            func=mybir.ActivationFunctionType.Identity,
                            bias=0.0, scale=1.0)
                        nc.gpsimd.dma_start(out=mu_dram.ap()[j:j + 1, :],
                                            in_=mstg[:])
                    if t == 20:
                        wgb = fcc.tile([P, 9, 4, P], BF16)
                        nc.sync.dma_start(out=wgb[:], in_=wg.ap())
                    if t == 24:
                        wbb_ = fcc.tile([P, 9, 4, P], BF16)
                        nc.sync.dma_start(out=wbb_[:], in_=wb.ap())

            # ---------------- Phase 2: finalize stats
            mv = stp.tile([P, 2], F32)
            nc.vector.bn_aggr(mv[:], stats[:])
            epsb = stp.tile([P, 1], F32)
            nc.vector.memset(epsb[:], EPS)
            rs = stp.tile([P, 1], F32)
            nc.scalar.activation(out=rs[:], in_=mv[:, 1:2],
                                 func=mybir.ActivationFunctionType.Sqrt,
                                 bias=epsb[:], scale=1.0)
            nc.vector.reciprocal(rs[:], rs[:])
            nbias = stp.tile([P, 1], F32)
            nc.vector.tensor_mul(nbias[:], mv[:, 0:1], rs[:])
            nc.vector.tensor_scalar_mul(nbias[:], nbias[:], -1.0)

            # ---------------- Phase 3: mu = relu(fc + bias), transpose to
            # the d-partitioned layout via 4 PE transposes
            nc.gpsimd.dma_start(out=mu_flat[:NCLS, :], in_=mu_dram.ap())
            nc.vector.tensor_add(mu_flat[:NCLS, :], mu_flat[:NCLS, :],
                                 bmub[:NCLS, :])
            nc.vector.tensor_scalar_max(mu_flat[:NCLS, :],
                                        mu_flat[:NCLS, :], 0.0)
            mu_fbf = fcc.tile([32, SL], BF16)
            nc.vector.memset(mu_fbf[:], 0.0)
            nc.vector.tensor_copy(mu_fbf[:NCLS, :], mu_flat[:NCLS, :])
            mu_bf = fcc.tile([P, 4, NCLS], BF16)
            psfc_cm.__exit__(None, None, None)
            with tc.tile_pool(name="pst", bufs=2, space="PSUM") as pst:
                for q in range(4):
                    psT = pst.tile([P, 32], BF16, tag="psT")
                    nc.tensor.transpose(psT[:], mu_fbf[:, q * P:(q + 1) * P],
                                        idnb[:])
                    nc.vector.tensor_copy(mu_bf[:, q, :], psT[:, :NCLS])

            # ---------------- Phase 4: effective weights
            with tc.tile_pool(name="pse", bufs=2, space="PSUM") as pse:
                for wsrcb, wdst in ((wgb, weff_g), (wbb_, weff_b)):
                    pe = pse.tile([NCLS, 9, P], F32, tag="pse")
                    for dd in range(9):
                        for q in range(4):
                            nc.tensor.matmul(pe[:, dd, :], lhsT=mu_bf[:, q, :],
                                             rhs=wsrcb[:, dd, q, :],
                                             start=(q == 0), stop=(q == 3))
                    for c in range(3):
                        src = pe[:, c:, :]
                        src = bass.AP(tensor=src.tensor, offset=src.offset,
                                      ap=[src.ap[0], [3 * P, 3], [1, P]])
                        nc.vector.tensor_copy(
                            wdst[32 * c:32 * c + NCLS, :, :], src)
            fcc_cm.__exit__(None, None, None)
            fcw_cm.__exit__(None, None, None)

            # ---------------- Phase 5: main conv + epilogue
            with (
                tc.tile_pool(name="psm", bufs=4, space="PSUM") as psm,
                tc.tile_pool(name="wk", bufs=3) as wk,
            ):
                for t in range(NT):
                    gps = psm.tile([P, 512], F32, tag="gps")
                    bps = psm.tile([P, 512], F32, tag="bps")
                    for ky in range(3):
                        nc.tensor.matmul(gps[:], lhsT=weff_g[:KREP, ky, :],
                                         rhs=rep_ap(t, ky, segb[:KREP]),
                                         start=(ky == 0), stop=False)
                    for i, (ky, kx) in enumerate(offs):
                        nc.tensor.matmul(gps[:], lhsT=wsgb[:, i, :],
                                         rhs=pix_ap(t, ky, kx, actv),
                                         start=False, stop=(i == 8))
                    for ky in range(3):
                        nc.tensor.matmul(bps[:], lhsT=weff_b[:KREP, ky, :],
                                         rhs=rep_ap(t, ky, segb[:KREP]),
                                         start=(ky == 0), stop=False)
                    for i, (ky, kx) in enumerate(offs):
                        nc.tensor.matmul(bps[:], lhsT=wsbb[:, i, :],
                                         rhs=pix_ap(t, ky, kx, actv),
                                         start=False, stop=(i == 8))
                    sl = slice(512 * t, 512 * (t + 1))
                    nrm = wk.tile([P, 512], F32, tag="nrm")
                    nc.scalar.activation(out=nrm[:], in_=xsl(t),
                                         func=mybir.ActivationFunctionType.Identity,
                                         bias=nbias[:], scale=rs[:])
                    g = wk.tile([P, 512], F32, tag="g")
                    nc.scalar.activation(out=g[:], in_=gps[:],
                                         func=mybir.ActivationFunctionType.Identity,
                                         bias=gbb[:], scale=1.0)
                    bb2 = wk.tile([P, 512], F32, tag="bb2")
                    nc.scalar.activation(out=bb2[:], in_=bps[:],
                                         func=mybir.ActivationFunctionType.Identity,
                                         bias=bbb[:], scale=1.0)
                    o = wk.tile([P, 512], F32, tag="o")
                    nc.vector.tensor_mul(o[:], nrm[:], g[:])
                    nc.vector.tensor_add(o[:], o[:], bb2[:])
                    nc.sync.dma_start(out=out.ap()[:, sl], in_=o[:])

    nc.compile()
    return nc


def _prep_inputs(x, segmap, style_codes, noise, noise_var, blending_gamma,
                 blending_beta, W_mu, b_mu, conv_gamma_w, conv_gamma_b,
                 conv_beta_w, conv_beta_b, sp_shared_w, sp_shared_b,
                 sp_gamma_w, sp_gamma_b, sp_beta_w, sp_beta_b):
    f32 = np.float32
    bf = ml_dtypes.bfloat16
    x = np.asarray(x, f32)
    segmap = np.asarray(segmap, f32)
    style_codes = np.asarray(style_codes, f32)
    noise = np.asarray(noise, f32)
    noise_var = np.asarray(noise_var, f32)
    W_mu = np.asarray(W_mu, f32)
    b_mu = np.asarray(b_mu, f32)

    ga = float(1.0 / (1.0 + np.exp(-f32(np.asarray(blending_gamma).reshape(-1)[0]))))
    ba = float(1.0 / (1.0 + np.exp(-f32(np.asarray(blending_beta).reshape(-1)[0]))))

    # padded one-hot segmap, replicated into 3 column-shifted copies on the
    # partition axis (kx baked into copy index), bf16 (exact: values 0/1)
    tmp = np.zeros((B, NCLS, HP, WP), bf)
    tmp[:, :, 1:H + 1, 1:W + 1] = segmap.astype(bf)
    tmp = tmp.reshape(B, NCLS, HP * WP)
    segp = np.zeros((B, P, HP * WP), bf)
    for c in range(3):
        segp[:, 32 * c:32 * c + NCLS, :HP * WP - c] = tmp[:, :, c:]

    # noise row-map per batch: nt[h, w] = noise[b, w, h, 0]
    ntm = noise[:, :, :, 0].transpose(0, 2, 1).reshape(B, 1, NPIX).astype(bf)

    # FC weights: wmu_t[j, p, kq, d] = W_mu[j, kq*128+p, d]
    wmu_t = np.ascontiguousarray(
        W_mu.reshape(NCLS, 4, P, SL).transpose(0, 2, 1, 3).astype(bf))
    # style_t[b][p, j, kq, 1] = style_codes[b, j, kq*128+p]
    style_t = style_codes.reshape(B, NCLS, 4, P).transpose(0, 3, 1, 2)[..., None].astype(bf)
    style_t = [np.ascontiguousarray(style_t[b]) for b in range(B)]
    # bmu_t[j, d] = b_mu[j, d], rows 19+ zero
    bmu_t = np.zeros((32, SL), f32)
    bmu_t[:NCLS] = b_mu
    idn_t = np.eye(32, dtype=bf)

    # big conv weights -> [dd, d, c] (blend factors folded in)
    wgt = (ga * conv_gamma_w.astype(f32)).transpose(2, 3, 1, 0).reshape(9, SL, C)
    wbt = (ba * conv_beta_w.astype(f32)).transpose(2, 3, 1, 0).reshape(9, SL, C)

    # SPADE weights; wsh replicated: row 32c+j, col ky = w[n, j, ky, c]
    wsh_t = np.zeros((P, 3, NHID), bf)
    _w = np.asarray(sp_shared_w, f32).transpose(1, 2, 3, 0)  # [j, ky, kx, n]
    for c in range(3):
        wsh_t[32 * c:32 * c + NCLS] = _w[:, :, c, :].astype(bf)
    wsg_t = ((1.0 - ga) * np.asarray(sp_gamma_w, f32)).transpose(1, 2, 3, 0).reshape(NHID, 9, C)
    wsb_t = ((1.0 - ba) * np.asarray(sp_beta_w, f32)).transpose(1, 2, 3, 0).reshape(NHID, 9, C)

    gbias_full = (1.0 + ga * np.asarray(conv_gamma_b, f32)
                  + (1.0 - ga) * np.asarray(sp_gamma_b, f32))
    bbias_full = (ba * np.asarray(conv_beta_b, f32)
                  + (1.0 - ba) * np.asarray(sp_beta_b, f32))
    bsh_t = np.asarray(sp_shared_b, f32).reshape(NHID, 1)

    in_maps = []
    for core in range(NCORES):
        b = core // 4
        cs = core % 4
        csl = slice(cs * P, (cs + 1) * P)
        m = {
            "x_s": np.ascontiguousarray(x[b, csl].reshape(P, NPIX)),
            "nt": ntm[b],
            "nv": np.ascontiguousarray(noise_var[csl].reshape(1, P).astype(bf)),
            "seg": np.ascontiguousarray(segp[b]),
            "wmu": wmu_t,
            "style": style_t[b],
            "bmu": bmu_t,
            "idn": idn_t,
            "wg": np.ascontiguousarray(
                wgt[:, :, csl].reshape(9, 4, P, P).transpose(2, 0, 1, 3).astype(bf)),
            "wb": np.ascontiguousarray(
                wbt[:, :, csl].reshape(9, 4, P, P).transpose(2, 0, 1, 3).astype(bf)),
            "wsh": wsh_t,
            "bsh": bsh_t,
            "wsg": np.ascontiguousarray(wsg_t[:, :, csl].astype(bf)),
            "wsb": np.ascontiguousarray(wsb_t[:, :, csl].astype(bf)),
            "gbias": np.ascontiguousarray(gbias_full[csl].reshape(P, 1)),
            "bbias": np.ascontiguousarray(bbias_full[csl].reshape(P, 1)),
        }
        in_maps.append(m)
    return in_maps


def kernel(**inputs):
    if "nc" not in _cache:
        _cache["nc"] = _build()
    nc = _cache["nc"]
    in_maps = _prep_inputs(**inputs)
    res = run_bass_kernel_spmd(nc, in_maps, core_ids=list(range(NCORES)))
    out = np.empty((B, C, H, W), np.float32)
    for core in range(NCORES):
        b = core // 4
        cs = core % 4
        out[b, cs * P:(cs + 1) * P] = np.asarray(
            res.results[core]["out"]).reshape(P, H, W)
    return out
